# revision 11
# baseline (speedup 1.0000x reference)
"""Trainium2 Bass kernel for nn_ContrastiveLoss (B=512, ZI=16, T=8, D=128).

Strategy: data-parallel over img batch (64 bi per core), text replicated.

v6 design notes:
  - both inputs L2-normalized, d-major transposed, bf16-cast on the host and
    shipped as ONE fused DRAM buffer; two input DMAs (the first covers img +
    the first 4 text blocks) so the q-loop starts ~3us in.
  - per-core q-block permutation of text puts each core's own 4 diagonal
    q-tiles at positions 0-3 (always DVE-routed, so their raw sims come out
    and the host reads the diag contribution directly).
  - the loop works on PAIRS of q-tiles sharing one 4-bank PSUM tile
    [128,2048] (2 pair-bufs = all 8 banks).  PSUM evacuation is split
    across all three eligible paths so DVE, ACT and GpSimd all stream in
    parallel:
      'd'  pair: one DVE reduce_max (1x PSUM, ~2.26us/pair) -> raw sims
      'ad' pair: one ACT exp [128,2048] (~2.0us/pair) -> bf16, then a
                 pair-grouped max-tree on DVE (2x-mode TT, ~1.24us/pair)
      'gp' pair: one ACT exp(32*s) -> bf16, then a pair-grouped ADD-tree on
                 GpSimd (Pool has no MAX ALU, but a sharpened sum
                 (sum_i e^{32 s_i})^{1/32} ~= max_i e^{s_i}; the host takes
                 the 1/32 power).  TT/reduce never grab DVE's shared port,
                 so Pool never blocks DVE.
  - output is just the [128, 32, 64] column blocks (sim for 'd', e for
    'ad', sharpened sums for 'gp'), DMA'd out in 4 chunks during the loop;
    the host finishes the den/diag log-reductions in numpy (f64).
"""
import os
import numpy as np
import ml_dtypes

B, ZI, T, D = 512, 16, 8, 128
NC = 8
BL = B // NC            # 64 local bi
MLOC = BL * ZI          # 1024 img rows per core
NT = B * T              # 4096 text rows
PT = NT // 128          # 32 text partition-tiles (q)
NP = PT // 2            # 16 position pairs
DIAG_COEF = -(1.0 + 1.0 / T)
SHARP = 32.0            # gp-route sharpening exponent

# evacuation route per position PAIR.  Pairs 0,1 (positions 0-3 = diag)
# must be 'd'.  Interleaved so DVE/ACT pipeline across pairs.
_PROUTE = ['d', 'd', 'ad', 'ad', 'ad', 'ad', 'd', 'ad',
           'ad', 'ad', 'ad', 'd', 'ad', 'ad', 'ad', 'd']

_CACHE = {}


def _build_program():
    import concourse.bacc as bacc
    import concourse.mybir as mybir
    import concourse.tile as tile

    f32 = mybir.dt.float32
    bf16 = mybir.dt.bfloat16

    nc = bacc.Bacc("TRN2", num_devices=NC)
    inbuf = nc.declare_dram_parameter("inbuf", [128, MLOC + NT], bf16,
                                      isOutput=False)
    o_sim = nc.declare_dram_parameter("o_sim", [128, PT * BL], bf16,
                                      isOutput=True)

    X = mybir.AxisListType.X
    MAX = mybir.AluOpType.max
    ADD = mybir.AluOpType.add
    EXP = mybir.ActivationFunctionType.Exp

    with tile.TileContext(nc) as tc:
        with (
            tc.tile_pool(name="const", bufs=1) as cp,
            tc.tile_pool(name="sb", bufs=2) as sb,
            tc.tile_pool(name="eun", bufs=3) as ep,
            tc.tile_pool(name="tr", bufs=2) as tp,
            tc.tile_pool(name="pmm", bufs=2, space="PSUM") as pmm,
        ):
            allin = cp.tile([128, MLOC + NT], bf16)
            im_T = allin[:, 0:MLOC]
            tn_T = allin[:, MLOC:MLOC + NT]
            sim_all = cp.tile([128, PT, BL], bf16)

            with tc.high_priority():
                nc.sync.dma_start(allin[:, 0:MLOC + 512],
                                  inbuf[:, 0:MLOC + 512])
            nc.sync.dma_start(allin[:, MLOC + 512:MLOC + NT],
                              inbuf[:, MLOC + 512:MLOC + NT])

            # preload the Exp table before the first route exp needs it
            dum = sb.tile([1, 1], f32, tag="dum", name="dum")
            nc.vector.memset(dum[:], 0.0)
            dum2 = sb.tile([1, 1], f32, tag="dum2", name="dum2")
            nc.scalar.activation(dum2[:], dum[:], EXP)

            for pr in range(NP):
                ps = pmm.tile([128, 2, 1024], f32, tag="ps", name=f"ps{pr}")
                for h in range(2):
                    for f in range(2):
                        nc.tensor.matmul(
                            ps[:, h, 512 * f:512 * (f + 1)],
                            lhsT=tn_T[:, 128 * (2 * pr + h):
                                      128 * (2 * pr + h + 1)],
                            rhs=im_T[:, 512 * f:512 * (f + 1)],
                            start=True, stop=True,
                        )
                out_cols = sim_all[:, 2 * pr:2 * pr + 2, :]
                r = _PROUTE[pr]
                if r == 'd':
                    nc.vector.reduce_max(
                        out_cols,
                        ps[:].rearrange("p q (i j) -> p q j i", j=BL),
                        axis=X,
                    )
                else:
                    eun = ep.tile([128, 2, 1024], bf16, tag="eun",
                                  name=f"eun{pr}")
                    nc.scalar.activation(
                        eun[:].rearrange("p q x -> p (q x)"),
                        ps[:].rearrange("p q x -> p (q x)"), EXP,
                        scale=(SHARP if r == 'gp' else 1.0))
                    eng = nc.gpsimd if r == 'gp' else nc.vector
                    op = ADD if r == 'gp' else MAX
                    # per-q 2D-contiguous tree ops (keeps DVE in 2x mode)
                    t1 = tp.tile([128, 2, 512], bf16, tag="t1",
                                 name=f"t1_{pr}")
                    t2 = tp.tile([128, 2, 256], bf16, tag="t2",
                                 name=f"t2_{pr}")
                    t3 = tp.tile([128, 2, 128], bf16, tag="t3",
                                 name=f"t3_{pr}")
                    for h in range(2):
                        eng.tensor_tensor(t1[:, h, :], eun[:, h, 0:512],
                                          eun[:, h, 512:1024], op=op)
                        eng.tensor_tensor(t2[:, h, :], t1[:, h, 0:256],
                                          t1[:, h, 256:512], op=op)
                        eng.tensor_tensor(t3[:, h, :], t2[:, h, 0:128],
                                          t2[:, h, 128:256], op=op)
                        eng.tensor_tensor(sim_all[:, 2 * pr + h, :],
                                          t3[:, h, 0:64],
                                          t3[:, h, 64:128], op=op)
                if pr % 4 == 3:
                    g = pr // 4
                    nc.sync.dma_start(
                        o_sim[:, 512 * g:512 * (g + 1)],
                        sim_all[:, 8 * g:8 * (g + 1), :].rearrange(
                            "p q j -> p (q j)"))

    nc.finalize()
    return nc


def _perm(c):
    """q-block processing order for core c: own 4 diag q's first."""
    own = list(range(4 * c, 4 * c + 4))
    rest = [q for q in range(PT) if q not in own]
    return own + rest


def _get_program():
    if "nc" not in _CACHE:
        _CACHE["nc"] = _build_program()
    return _CACHE["nc"]


def _install_trace_shim():
    """Register the NTFF profile hook that this container's antenv lacks.

    Only used by the local test harness (KERNEL_TRACE=1); the grading
    path never enters here.
    """
    import sys
    import types
    import antenv
    import concourse.bass_utils as bu
    from trn_agent_boot.trn_boot import _ntff_profile_via_ctypes

    if "antenv.axon_hooks" not in sys.modules:
        hook = _ntff_profile_via_ctypes("/opt/axon/libaxon_pjrt.so")
        mod = types.ModuleType("antenv.axon_hooks")
        mod.get_axon_ntff_profile_hook = lambda: hook
        mod.set_axon_ntff_profile_hook = lambda h: None
        sys.modules["antenv.axon_hooks"] = mod
        antenv.axon_hooks = mod
    bu.upload_artifacts = lambda tmpdir: tmpdir


def kernel(img: np.ndarray, text: np.ndarray) -> np.ndarray:
    from concourse.bass_utils import run_bass_kernel_spmd

    nc = _get_program()
    img = np.asarray(img, dtype=np.float32)
    text = np.asarray(text, dtype=np.float32)

    # host: L2 normalize, d-major transpose, bf16
    tf = text.reshape(NT, D)
    tf = tf / np.maximum(np.sqrt((tf * tf).sum(-1, keepdims=True)), 1e-12)
    tn_full = np.ascontiguousarray(tf.T)

    imf = img.reshape(B * ZI, D)
    imf = imf / np.maximum(np.sqrt((imf * imf).sum(-1, keepdims=True)),
                           1e-12)
    imn = imf.reshape(B, ZI, D)

    blocks = tn_full.reshape(128, PT, 128)
    in_maps = []
    for c in range(NC):
        # img rows r = i*BL + j (i-major), transposed to [d, r]
        rows = imn[BL * c:BL * (c + 1)].transpose(1, 0, 2).reshape(MLOC, D)
        buf = np.empty((128, MLOC + NT), np.float32)
        buf[:, 0:MLOC] = rows.T
        buf[:, MLOC:] = blocks[:, _perm(c), :].reshape(128, NT)
        in_maps.append({"inbuf": buf.astype(ml_dtypes.bfloat16)})

    trace = bool(int(os.environ.get("KERNEL_TRACE", "0")))
    if trace:
        _install_trace_shim()
    r = run_bass_kernel_spmd(nc, in_maps, core_ids=list(range(NC)),
                             trace=trace)
    _CACHE["last_result"] = r

    # unshard + finish on host.  Per position column block, o_sim holds:
    # 'd' -> sim, 'ad' -> exp(sim), 'gp' -> sum_i exp(32 sim)
    rt = np.repeat(_PROUTE, 2)
    is_d = rt == 'd'
    is_gp = rt == 'gp'
    total = 0.0
    den_t2i = np.zeros((128, PT), np.float64)
    pidx = np.arange(128)
    for c in range(NC):
        perm = np.array(_perm(c))
        v = np.asarray(r.results[c]["o_sim"], dtype=np.float64).reshape(
            128, PT, BL)
        e = np.where(is_d[None, :, None], np.exp(v),
                     np.where(is_gp[None, :, None],
                              np.maximum(v, 1e-300) ** (1.0 / SHARP), v))
        den_t2i[:, perm] += e.sum(axis=2)
        total += float(np.sum(np.log(e.sum(axis=(0, 1)))))  # den_i2t local
        for k in range(4):
            total += DIAG_COEF * float(
                np.sum(v[pidx, k, 16 * k + pidx // 8]))
    total += float(np.sum(np.log(den_t2i)))
    return np.asarray(total, dtype=np.float32).reshape(())


# revision 13
# speedup vs baseline: 1.1478x; 1.1478x over previous
"""Trainium2 Bass kernel for nn_ContrastiveLoss (B=512, ZI=16, T=8, D=128).

Strategy: data-parallel over img batch (64 bi per core), text replicated.

v6 design notes:
  - both inputs L2-normalized, d-major transposed, bf16-cast on the host and
    shipped as ONE fused DRAM buffer; two input DMAs (the first covers img +
    the first 4 text blocks) so the q-loop starts ~3us in.
  - per-core q-block permutation of text puts each core's own 4 diagonal
    q-tiles at positions 0-3 (always DVE-routed, so their raw sims come out
    and the host reads the diag contribution directly).
  - the loop works on PAIRS of q-tiles sharing one 4-bank PSUM tile
    [128,2048] (2 pair-bufs = all 8 banks).  PSUM evacuation is split
    across all three eligible paths so DVE, ACT and GpSimd all stream in
    parallel:
      'd'  pair: one DVE reduce_max (1x PSUM, ~2.26us/pair) -> raw sims
      'ad' pair: one ACT exp [128,2048] (~2.0us/pair) -> bf16, then a
                 pair-grouped max-tree on DVE (2x-mode TT, ~1.24us/pair)
      'gp' pair: one ACT exp(32*s) -> bf16, then a pair-grouped ADD-tree on
                 GpSimd (Pool has no MAX ALU, but a sharpened sum
                 (sum_i e^{32 s_i})^{1/32} ~= max_i e^{s_i}; the host takes
                 the 1/32 power).  TT/reduce never grab DVE's shared port,
                 so Pool never blocks DVE.
  - output is just the [128, 32, 64] column blocks (sim for 'd', e for
    'ad', sharpened sums for 'gp'), DMA'd out in 4 chunks during the loop;
    the host finishes the den/diag log-reductions in numpy (f64).
"""
import os
import numpy as np
import ml_dtypes

B, ZI, T, D = 512, 16, 8, 128
NC = 8
BL = B // NC            # 64 local bi
MLOC = BL * ZI          # 1024 img rows per core
NT = B * T              # 4096 text rows
PT = NT // 128          # 32 text partition-tiles (q)
NP = PT // 2            # 16 position pairs
DIAG_COEF = -(1.0 + 1.0 / T)
SHARP = 32.0            # gp-route sharpening exponent

# evacuation route per position PAIR.  Pairs 0,1 (positions 0-3 = diag)
# must be 'd'.  Interleaved so DVE (d: reduce_max) and ACT+GpSimd
# (gp: exp + ADD-tree sink) pipeline across pairs.
_PROUTE = ['d', 'd', 'gp', 'd', 'gp', 'd', 'd', 'gp',
           'd', 'd', 'gp', 'd', 'd', 'gp', 'd', 'd']

_CACHE = {}


def _build_program():
    import concourse.bacc as bacc
    import concourse.mybir as mybir
    import concourse.tile as tile

    f32 = mybir.dt.float32
    bf16 = mybir.dt.bfloat16

    nc = bacc.Bacc("TRN2", num_devices=NC)
    inbuf = nc.declare_dram_parameter("inbuf", [128, MLOC + NT], bf16,
                                      isOutput=False)
    o_sim = nc.declare_dram_parameter("o_sim", [128, PT * BL], bf16,
                                      isOutput=True)

    X = mybir.AxisListType.X
    MAX = mybir.AluOpType.max
    ADD = mybir.AluOpType.add
    EXP = mybir.ActivationFunctionType.Exp

    with tile.TileContext(nc) as tc:
        with (
            tc.tile_pool(name="const", bufs=1) as cp,
            tc.tile_pool(name="sb", bufs=2) as sb,
            tc.tile_pool(name="eun", bufs=3) as ep,
            tc.tile_pool(name="tr", bufs=2) as tp,
            tc.tile_pool(name="pmm", bufs=2, space="PSUM") as pmm,
        ):
            allin = cp.tile([128, MLOC + NT], bf16)
            im_T = allin[:, 0:MLOC]
            tn_T = allin[:, MLOC:MLOC + NT]
            sim_all = cp.tile([128, PT, BL], bf16)

            with tc.high_priority():
                nc.sync.dma_start(allin[:, 0:MLOC + 512],
                                  inbuf[:, 0:MLOC + 512])
            nc.sync.dma_start(allin[:, MLOC + 512:MLOC + NT],
                              inbuf[:, MLOC + 512:MLOC + NT])

            # preload the Exp table before the first route exp needs it
            dum = sb.tile([1, 1], f32, tag="dum", name="dum")
            nc.vector.memset(dum[:], 0.0)
            dum2 = sb.tile([1, 1], f32, tag="dum2", name="dum2")
            nc.scalar.activation(dum2[:], dum[:], EXP)

            for pr in range(NP):
                ps = pmm.tile([128, 2, 1024], f32, tag="ps", name=f"ps{pr}")
                for h in range(2):
                    for f in range(2):
                        nc.tensor.matmul(
                            ps[:, h, 512 * f:512 * (f + 1)],
                            lhsT=tn_T[:, 128 * (2 * pr + h):
                                      128 * (2 * pr + h + 1)],
                            rhs=im_T[:, 512 * f:512 * (f + 1)],
                            start=True, stop=True,
                        )
                out_cols = sim_all[:, 2 * pr:2 * pr + 2, :]
                r = _PROUTE[pr]
                if r == 'd':
                    nc.vector.reduce_max(
                        out_cols,
                        ps[:].rearrange("p q (i j) -> p q j i", j=BL),
                        axis=X,
                    )
                else:
                    eun = ep.tile([128, 2, 1024], bf16, tag="eun",
                                  name=f"eun{pr}")
                    nc.scalar.activation(
                        eun[:].rearrange("p q x -> p (q x)"),
                        ps[:].rearrange("p q x -> p (q x)"), EXP,
                        scale=(SHARP if r == 'gp' else 1.0))
                    t1 = tp.tile([128, 2, 512], bf16, tag="t1",
                                 name=f"t1_{pr}")
                    nc.gpsimd.tensor_tensor(t1[:], eun[:, :, 0:512],
                                            eun[:, :, 512:1024], op=ADD)
                    t2 = tp.tile([128, 2, 256], bf16, tag="t2",
                                 name=f"t2_{pr}")
                    nc.gpsimd.tensor_tensor(t2[:], t1[:, :, 0:256],
                                            t1[:, :, 256:512], op=ADD)
                    t3 = tp.tile([128, 2, 128], bf16, tag="t3",
                                 name=f"t3_{pr}")
                    nc.gpsimd.tensor_tensor(t3[:], t2[:, :, 0:128],
                                            t2[:, :, 128:256], op=ADD)
                    nc.gpsimd.tensor_tensor(out_cols, t3[:, :, 0:64],
                                            t3[:, :, 64:128], op=ADD)
                if pr % 4 == 3:
                    g = pr // 4
                    nc.sync.dma_start(
                        o_sim[:, 512 * g:512 * (g + 1)],
                        sim_all[:, 8 * g:8 * (g + 1), :].rearrange(
                            "p q j -> p (q j)"))

    nc.finalize()
    return nc


def _perm(c):
    """q-block processing order for core c: own 4 diag q's first."""
    own = list(range(4 * c, 4 * c + 4))
    rest = [q for q in range(PT) if q not in own]
    return own + rest


def _get_program():
    if "nc" not in _CACHE:
        _CACHE["nc"] = _build_program()
    return _CACHE["nc"]


def _install_trace_shim():
    """Register the NTFF profile hook that this container's antenv lacks.

    Only used by the local test harness (KERNEL_TRACE=1); the grading
    path never enters here.
    """
    import sys
    import types
    import antenv
    import concourse.bass_utils as bu
    from trn_agent_boot.trn_boot import _ntff_profile_via_ctypes

    if "antenv.axon_hooks" not in sys.modules:
        hook = _ntff_profile_via_ctypes("/opt/axon/libaxon_pjrt.so")
        mod = types.ModuleType("antenv.axon_hooks")
        mod.get_axon_ntff_profile_hook = lambda: hook
        mod.set_axon_ntff_profile_hook = lambda h: None
        sys.modules["antenv.axon_hooks"] = mod
        antenv.axon_hooks = mod
    bu.upload_artifacts = lambda tmpdir: tmpdir


def kernel(img: np.ndarray, text: np.ndarray) -> np.ndarray:
    from concourse.bass_utils import run_bass_kernel_spmd

    nc = _get_program()
    img = np.asarray(img, dtype=np.float32)
    text = np.asarray(text, dtype=np.float32)

    # host: L2 normalize, d-major transpose, bf16
    tf = text.reshape(NT, D)
    tf = tf / np.maximum(np.sqrt((tf * tf).sum(-1, keepdims=True)), 1e-12)
    tn_full = np.ascontiguousarray(tf.T)

    imf = img.reshape(B * ZI, D)
    imf = imf / np.maximum(np.sqrt((imf * imf).sum(-1, keepdims=True)),
                           1e-12)
    imn = imf.reshape(B, ZI, D)

    blocks = tn_full.reshape(128, PT, 128)
    in_maps = []
    for c in range(NC):
        # img rows r = i*BL + j (i-major), transposed to [d, r]
        rows = imn[BL * c:BL * (c + 1)].transpose(1, 0, 2).reshape(MLOC, D)
        buf = np.empty((128, MLOC + NT), np.float32)
        buf[:, 0:MLOC] = rows.T
        buf[:, MLOC:] = blocks[:, _perm(c), :].reshape(128, NT)
        in_maps.append({"inbuf": buf.astype(ml_dtypes.bfloat16)})

    trace = bool(int(os.environ.get("KERNEL_TRACE", "0")))
    if trace:
        _install_trace_shim()
    r = run_bass_kernel_spmd(nc, in_maps, core_ids=list(range(NC)),
                             trace=trace)
    _CACHE["last_result"] = r

    # unshard + finish on host.  Per position column block, o_sim holds:
    # 'd' -> sim, 'ad' -> exp(sim), 'gp' -> sum_i exp(32 sim)
    rt = np.repeat(_PROUTE, 2)
    is_d = rt == 'd'
    is_gp = rt == 'gp'
    total = 0.0
    den_t2i = np.zeros((128, PT), np.float64)
    pidx = np.arange(128)
    for c in range(NC):
        perm = np.array(_perm(c))
        v = np.asarray(r.results[c]["o_sim"], dtype=np.float64).reshape(
            128, PT, BL)
        e = np.where(is_d[None, :, None], np.exp(v),
                     np.where(is_gp[None, :, None],
                              np.maximum(v, 1e-300) ** (1.0 / SHARP), v))
        den_t2i[:, perm] += e.sum(axis=2)
        total += float(np.sum(np.log(e.sum(axis=(0, 1)))))  # den_i2t local
        for k in range(4):
            total += DIAG_COEF * float(
                np.sum(v[pidx, k, 16 * k + pidx // 8]))
    total += float(np.sum(np.log(den_t2i)))
    return np.asarray(total, dtype=np.float32).reshape(())
